# revision 1
# baseline (speedup 1.0000x reference)
"""COIL sparse-attention scoring kernel for 8 Trainium2 NeuronCores.

Strategy
--------
Shard the doc axis (Bd=128) across the 8 cores (16 docs each); qry tensors are
replicated. The exact-token-match mask is folded INTO the matmul: each token id
(vocab 1000) is encoded as three base-10 digit one-hots scaled by ALPHA=32 and
appended to the reps. Then

    v[qs, ct] = <qry_ext[qs], doc_ext[ct]> = S[qs, ct] + 1024 * match_digits

where match_digits == 3 iff the ids are equal, so

    tok[qs, c] = relu(max_t v[qs, c, t] - 3072)

reproduces the reference masked-max exactly (non-match scores sit below 2100,
matches above 3000). The qry reps are split hi/lo in bf16 (3 cross terms) so
the matmul runs at full bf16 rate with ~fp32 accuracy: K = 3*32 + 30 = 126.

Per core: 32 q-tiles of 128 q-positions; each q-tile is one [126,128]x[126,2048]
matmul into PSUM [128, 2048]. The per-doc max over the 128 doc tokens is split
between the DVE (direct tensor_reduce from PSUM) and a ScalarE relu-convert to
fp16 followed by a DVE tensor_tensor max tree at 2x rate. The sum over query
positions is a ones-vector matmul on the PE (partition-dim sum). CLS scores,
the tiny 4-way q-tile fold, and the final max over the 8 query chunks are done
on host (a few thousand elements).
"""

import os
import numpy as np
import ml_dtypes

Bq, Sq, Bd, Sd, D, Dc = 8, 512, 128, 128, 32, 768
NCORES = 8
BD_PER = Bd // NCORES          # 16 docs per core
K_EXT = 126                    # 32*3 rep dims + 30 one-hot dims
SQF = Bq * Sq                  # 4096 query positions
DCOL = BD_PER * Sd             # 2048 doc tokens per core
NQT = SQF // 128               # 32 q-tiles
ALPHA = 32.0
OFF = 3.0 * ALPHA * ALPHA      # 3072: offset of a full 3-digit match
# q-tile qt goes to the DVE-direct path iff qt % DIRECT_PERIOD == DIRECT_PERIOD-1;
# the rest go ScalarE-relu-fp16 -> DVE max tree. Whole-tile assignment keeps each
# PSUM tile single-reader (fewer semaphore waits).
DIRECT_PERIOD = int(os.environ.get("KERNEL_DIRECT_PERIOD", "4"))
TREE_LEVELS = int(os.environ.get("KERNEL_TREE_LEVELS", "3"))
# 6 warm-up MMs (~3.8us busy) sits right at the HAM 3.4us flip threshold and
# is bimodal run-to-run (71.6 vs 75.5 measured); 12 is ~0.5us slower at best
# but stable across runs.
WARMUP_MMS = int(os.environ.get("KERNEL_WARMUP_MMS", "12"))
BRIDGE_MMS = int(os.environ.get("KERNEL_BRIDGE_MMS", "0"))
# prune query positions whose token id does not appear in this core's doc
# slab (they contribute exactly 0): compact to NQT_PRUNED q-tiles per core
PRUNE = os.environ.get("KERNEL_PRUNE", "1") == "1"
NQT_PRUNED = int(os.environ.get("KERNEL_NQT_PRUNED", "29"))
# fraction knob: tree q-tiles where GPSIMD runs the first max-tree level
# instead of the DVE. Disabled: this walrus build rejects TensorTensor on
# the Pool engine ("Instruction engine check failed").
GPSIMD_TT1_MOD = int(os.environ.get("KERNEL_GPSIMD_TT1_MOD", "0"))

_CACHE = {}


def _bf16(x):
    return x.astype(ml_dtypes.bfloat16)


def _onehot_digits(ids):
    """ids [N] int in [0,1000) -> [N,30] base-10 digit one-hot (float32)."""
    n = ids.shape[0]
    H = np.zeros((n, 30), dtype=np.float32)
    r = np.arange(n)
    H[r, ids % 10] = 1.0
    H[r, 10 + (ids // 10) % 10] = 1.0
    H[r, 20 + ids // 100] = 1.0
    return H


def _build_qry_ext(qry_reps, qry_input_ids, qry_attention_mask):
    q = np.asarray(qry_reps, np.float32).reshape(SQF, D)
    ids = np.asarray(qry_input_ids, np.int64).reshape(SQF)
    q_hi = _bf16(q).astype(np.float32)
    q_lo = _bf16(q - q_hi).astype(np.float32)
    H = ALPHA * _onehot_digits(ids)
    ext = np.concatenate([q_hi, q_lo, q_hi, H], axis=1)  # [SQF, 126]
    # rows that must contribute 0: CLS (s=0), SEP (last attended pos), mask==0
    mask = np.asarray(qry_attention_mask, np.int64).copy()
    sep = mask.sum(axis=1) - 1
    mask[np.arange(Bq), sep] = 0
    mask[:, 0] = 0
    ext *= mask.reshape(SQF, 1).astype(np.float32)
    return np.ascontiguousarray(_bf16(ext).T)  # [126, SQF]


def _build_doc_ext(doc_reps, doc_input_ids):
    d = np.asarray(doc_reps, np.float32).reshape(-1, D)
    ids = np.asarray(doc_input_ids, np.int64).reshape(-1)
    d_hi = _bf16(d).astype(np.float32)
    d_lo = _bf16(d - d_hi).astype(np.float32)
    H = ALPHA * _onehot_digits(ids)
    ext = np.concatenate([d_hi, d_hi, d_lo, H], axis=1)  # [N, 126]
    return np.ascontiguousarray(_bf16(ext).T)  # [126, N]


_LDW_PATCHED = False


def _patch_ldw_opt():
    """bir_verify_and_optimise hardcodes --enable-ldw-opt=false, which makes
    walrus emit one LDWEIGHTS per matmul even when the stationary operand is
    unchanged (4x redundant here). Append =true (last flag wins)."""
    global _LDW_PATCHED
    # Tile emits standalone InstLdweights, which walrus's ldw-opt rejects;
    # keep this opt-in for experiments only.
    if _LDW_PATCHED or not os.environ.get("KERNEL_LDW_OPT"):
        return
    import concourse.bass_utils as bu

    orig = bu.get_walrus_args

    def patched(*a, **k):
        return orig(*a, **k) + ["--enable-ldw-opt=true"]

    bu.get_walrus_args = patched
    _LDW_PATCHED = True


def _split_multi_waits(nc, mybir):
    """This container's walrus accepts only ONE sync-wait per instruction
    ("Too many sync wait commands"). Hoist extra waits into standalone
    EventSemaphore instructions on the same engine right before the offender
    (the sequencer blocks on each in order — semantically identical)."""
    n = 0
    for func in nc.m.functions:
        for bb in func.blocks:
            out = []
            for inst in bb.instructions:
                si = inst.sync_info
                if si is not None and len(si.on_wait) > 1:
                    waits = list(si.on_wait)
                    for w in waits[:-1]:
                        n += 1
                        out.append(
                            mybir.InstEventSemaphore(
                                name=f"W-{inst.name}-{n}",
                                engine=inst.engine,
                                ins=[],
                                outs=[],
                                debug=inst.debug,
                                sync_info=mybir.SyncInfo(
                                    on_wait=[w], on_update=[]
                                ),
                            )
                        )
                    inst.sync_info = mybir.SyncInfo(
                        on_wait=[waits[-1]], on_update=list(si.on_update)
                    )
                out.append(inst)
            bb.instructions = out
    return n


def _groups(nqt):
    """Final-sum groups: up to 8 q-tiles share one selector matmul (the
    off-diagonal blocks of the [8G, 16G] product are computed but unused)."""
    return [range(g, min(g + 8, nqt)) for g in range(0, nqt, 8)]


def _build_nc(direct_period, tree_levels, nqt):
    import concourse.bass as bass
    import concourse.mybir as mybir
    import concourse.tile as tile
    from concourse.bass import ts

    bf16, f16, f32 = mybir.dt.bfloat16, mybir.dt.float16, mybir.dt.float32
    nc = bass.Bass("TRN2", target_bir_lowering=False, debug=False)
    sqf = nqt * 128
    qryT = nc.dram_tensor("qryT", [K_EXT, sqf], bf16, kind="ExternalInput").ap()
    docT = nc.dram_tensor("docT", [K_EXT, DCOL], bf16, kind="ExternalInput").ap()
    selT = nc.dram_tensor("selT", [128, 8 * nqt], f32, kind="ExternalInput").ap()
    out = nc.dram_tensor("out", [64, 16 * nqt], f32, kind="ExternalOutput").ap()

    phase = int(os.environ.get("KERNEL_DIRECT_PHASE", "0"))
    is_direct = [
        direct_period > 0 and qt % direct_period == phase % direct_period
        for qt in range(nqt)
    ]
    n_direct = sum(is_direct)
    with tile.TileContext(nc) as tc:
        with (
            tc.tile_pool(name="inp", bufs=1) as inp,
            tc.tile_pool(name="psum", bufs=2, space="PSUM") as psum,
            tc.tile_pool(name="stage", bufs=3) as stp,
            tc.tile_pool(name="tree", bufs=2) as trp,
            tc.tile_pool(name="accp", bufs=1) as accp,
        ):
            # PE warm-up: ~3.5us of junk matmuls during the DMA head so the
            # HAM clock-gate reaches 8/8 before the real work starts
            scratch = inp.tile([K_EXT, 512], bf16)
            nc.vector.memset(scratch[:], 0.0)
            wps = psum.tile([128, 512], f32, tag="score")
            for _ in range(WARMUP_MMS):
                nc.tensor.matmul(
                    wps[:], scratch[:, 0:128], scratch[:], start=True, stop=True
                )

            # doc chunk 0 + qry chunk 0 first so q-tile 0 can start early
            qry_sb = inp.tile([K_EXT, sqf], bf16)
            doc_sb = inp.tile([K_EXT, DCOL], bf16)
            sel_sb = inp.tile([128, 8 * nqt], f32)
            # first chunks split across the HWDGE (sync) and SWDGE (gpsimd)
            # queues so they land in parallel instead of serializing
            nc.sync.dma_start(doc_sb[:, ts(0, 512)], docT[:, ts(0, 512)])
            nc.gpsimd.dma_start(qry_sb[:, ts(0, 512)], qryT[:, ts(0, 512)])
            nc.sync.dma_start(doc_sb[:, ts(1, 512)], docT[:, ts(1, 512)])
            nc.gpsimd.dma_start(doc_sb[:, ts(2, 512)], docT[:, ts(2, 512)])
            nc.sync.dma_start(doc_sb[:, ts(3, 512)], docT[:, ts(3, 512)])
            for off in range(512, sqf, 512):
                w = min(512, sqf - off)
                nc.sync.dma_start(qry_sb[:, off : off + w], qryT[:, off : off + w])
            nc.sync.dma_start(sel_sb[:], selT[:])

            accum = accp.tile([128, 16 * nqt], f32)
            draw = accp.tile([128, 16 * max(n_direct, 1)], f32)
            negoff = accp.tile([128, 1], f32)
            nc.vector.memset(negoff[:], -OFF)

            di = 0
            for qt in range(nqt):
                ps = psum.tile([128, DCOL], f32, tag="score")
                for j in range(DCOL // 512):
                    nc.tensor.matmul(
                        ps[:, ts(j, 512)],
                        qry_sb[:, ts(qt, 128)],
                        doc_sb[:, ts(j, 512)],
                        start=True,
                        stop=True,
                    )
                if is_direct[qt]:
                    # whole tile on DVE straight from PSUM (raw v scale),
                    # then tok = max(raw, OFF) - OFF into the accum cols
                    nc.vector.reduce_max(
                        draw[:, di * 16 : (di + 1) * 16],
                        ps[:].rearrange("p (c t) -> p c t", t=Sd),
                        axis=mybir.AxisListType.X,
                    )
                    nc.vector.tensor_scalar(
                        accum[:, qt * 16 : (qt + 1) * 16],
                        draw[:, di * 16 : (di + 1) * 16],
                        OFF,
                        -OFF,
                        mybir.AluOpType.max,
                        mybir.AluOpType.add,
                    )
                    di += 1
                else:
                    # fp16 relu(v - OFF) on ScalarE; tree then maxes toks
                    st = stp.tile([128, BD_PER * Sd], f16, tag="stage")
                    nc.scalar.activation(
                        st[:],
                        ps[:],
                        mybir.ActivationFunctionType.Relu,
                        bias=negoff[:],
                    )
                    cur, width = st, Sd
                    for lev in range(tree_levels):
                        nxt = trp.tile([128, BD_PER * width // 2], f16, tag=f"t{lev}")
                        cv = cur[:].rearrange("p (c t) -> p c t", t=width)
                        eng = (
                            nc.gpsimd
                            if (
                                lev == 0
                                and GPSIMD_TT1_MOD > 0
                                and qt % GPSIMD_TT1_MOD == 0
                            )
                            else nc.vector
                        )
                        eng.tensor_max(
                            nxt[:].rearrange("p (c t) -> p c t", t=width // 2),
                            cv[:, :, 0 : width // 2],
                            cv[:, :, width // 2 : width],
                        )
                        cur, width = nxt, width // 2
                    nc.vector.reduce_max(
                        accum[:, qt * 16 : (qt + 1) * 16],
                        cur[:].rearrange("p (c t) -> p c t", t=width),
                        axis=mybir.AxisListType.X,
                    )
            # a few junk matmuls with late priority: the scheduler runs them
            # when the PE idles after the last q-tile, keeping the HAM clock
            # warm for the final partition-sum matmuls
            for _ in range(BRIDGE_MMS):
                bp = psum.tile([128, 512], f32, tag="score")
                nc.tensor.matmul(
                    bp[:], scratch[:, 0:128], scratch[:], start=True, stop=True
                )
            # per-q partition sums: for each group of up to 8 q-tiles, one
            # matmul with the q-membership selector as the stationary operand;
            # only the [8,16] diagonal blocks are used (host slices them out)
            osb = accp.tile([64, 16 * nqt], f32)
            nc.vector.memset(osb[:], 0.0)
            for g, grp in enumerate(_groups(nqt)):
                qts = list(grp)
                gn = len(qts)
                c0 = qts[0] * 16
                fin = psum.tile([8 * gn, 16 * gn], f32, tag="score")
                nc.tensor.matmul(
                    fin[:],
                    sel_sb[:, qts[0] * 8 : (qts[-1] + 1) * 8],
                    accum[:, c0 : c0 + 16 * gn],
                    start=True,
                    stop=True,
                )
                if g % 2 == 0:
                    nc.vector.tensor_copy(osb[0 : 8 * gn, c0 : c0 + 16 * gn], fin[:])
                else:
                    nc.scalar.copy(osb[0 : 8 * gn, c0 : c0 + 16 * gn], fin[:])
            nc.sync.dma_start(out[:], osb[:])
    _split_multi_waits(nc, mybir)
    return nc


def _get_nc(nqt):
    _patch_ldw_opt()
    key = (
        DIRECT_PERIOD,
        TREE_LEVELS,
        nqt,
        os.environ.get("KERNEL_DIRECT_PHASE", "0"),
    )
    if key not in _CACHE:
        _CACHE[key] = _build_nc(key[0], key[1], nqt)
    return _CACHE[key]


def _qry_row_mask(inputs):
    """[Bq, Sq] bool: rows that can contribute (attended, not CLS/SEP)."""
    mask = np.asarray(inputs["qry_attention_mask"], np.int64).copy()
    sep = mask.sum(axis=1) - 1
    mask[np.arange(Bq), sep] = 0
    mask[:, 0] = 0
    return mask.astype(bool)


def _prepare_in_maps(inputs):
    qT_full = _build_qry_ext(
        inputs["qry_reps"], inputs["qry_input_ids"], inputs["qry_attention_mask"]
    )  # [K_EXT, SQF] bf16
    doc_reps = np.asarray(inputs["doc_reps"], np.float32)
    doc_ids = np.asarray(inputs["doc_input_ids"], np.int64)
    qry_ids = np.asarray(inputs["qry_input_ids"], np.int64).reshape(SQF)
    row_ok = _qry_row_mask(inputs).reshape(SQF)
    qpos_q = np.repeat(np.arange(Bq), Sq)  # q index of each row

    nqt = NQT
    sels = None
    if PRUNE:
        sels = []
        for core in range(NCORES):
            sl = slice(core * BD_PER, (core + 1) * BD_PER)
            vocab = np.zeros(1000, dtype=bool)
            vocab[doc_ids[sl].reshape(-1)] = True
            keep = row_ok & vocab[qry_ids]
            sels.append(np.nonzero(keep)[0])
        if max(len(s) for s in sels) <= NQT_PRUNED * 128:
            nqt = NQT_PRUNED
        else:  # fallback: shapes must be compile-time fixed
            sels = None

    in_maps = []
    sqf = nqt * 128
    for core in range(NCORES):
        sl = slice(core * BD_PER, (core + 1) * BD_PER)
        dT = _build_doc_ext(doc_reps[sl], doc_ids[sl])
        if sels is not None:
            rows = sels[core]
            qT = np.zeros((K_EXT, sqf), dtype=qT_full.dtype)
            qT[:, : len(rows)] = qT_full[:, rows]
            qrow = qpos_q[rows]
        else:
            qT = qT_full
            qrow = qpos_q
        # selector: sel[p, qt*8+m] = 1 iff row qt*128+p belongs to query m
        sel = np.zeros((128, 8 * nqt), dtype=np.float32)
        for qt in range(nqt):
            seg = qrow[qt * 128 : (qt + 1) * 128]
            sel[np.arange(len(seg)), qt * 8 + seg] = 1.0
        in_maps.append({"qryT": qT, "docT": dT, "selT": sel})
    return in_maps, nqt


def _assemble(inputs, results, nqt):
    toks = np.zeros((Bq, Bd), dtype=np.float32)
    for core in range(NCORES):
        osb = np.asarray(results[core]["out"], np.float32)  # [64, 16*nqt]
        part = np.zeros((Bq, BD_PER), dtype=np.float32)
        for g, grp in enumerate(_groups(nqt)):
            for tl, qt in enumerate(grp):
                part += osb[8 * tl : 8 * tl + 8, qt * 16 : (qt + 1) * 16]
        toks[:, core * BD_PER : (core + 1) * BD_PER] = part
    cls = np.asarray(inputs["qry_cls"], np.float32) @ np.asarray(
        inputs["doc_cls"], np.float32
    ).T
    scores = toks + cls
    return scores.max(axis=0).reshape(-1).astype(np.float32)


def _ensure_ntff_hook():
    """This container's antenv lacks axon_hooks; synthesize the module and
    register the ctypes-based NTFF profile hook so trace=True works."""
    import sys
    import types

    if "antenv.axon_hooks" in sys.modules:
        return
    mod = types.ModuleType("antenv.axon_hooks")
    state = {"hook": None}
    mod.set_axon_ntff_profile_hook = lambda h: state.__setitem__("hook", h)
    mod.get_axon_ntff_profile_hook = lambda: state["hook"]
    sys.modules["antenv.axon_hooks"] = mod
    try:
        import antenv

        antenv.axon_hooks = mod
    except ImportError:
        pass
    try:
        from trn_agent_boot.trn_boot import _ntff_profile_via_ctypes

        mod.set_axon_ntff_profile_hook(
            _ntff_profile_via_ctypes("/opt/axon/libaxon_pjrt.so")
        )
    except Exception:
        pass


def run(inputs, trace=False, **kwargs):
    """Run on the 8 NeuronCores; returns (output, BassKernelResults)."""
    from concourse.bass_utils import run_bass_kernel_spmd

    if trace:
        _ensure_ntff_hook()
    in_maps, nqt = _prepare_in_maps(inputs)
    nc = _get_nc(nqt)
    res = run_bass_kernel_spmd(
        nc, in_maps, core_ids=list(range(NCORES)), trace=trace, **kwargs
    )
    return _assemble(inputs, res.results, nqt), res


def kernel(**inputs) -> np.ndarray:
    out, _ = run(inputs)
    return out



# revision 4
# speedup vs baseline: 2.5716x; 2.5716x over previous
"""COIL sparse-attention scoring kernel for 8 Trainium2 NeuronCores (v2).

Strategy
--------
Shard the doc axis (Bd=128) across the 8 cores (16 docs each); qry tensors are
replicated. Exploit the match sparsity: a query position can only score against
doc tokens with the SAME token id, so the full [4096 x 2048] per-core score
matrix is ~99.6% irrelevant.

Host-side index prep (cheap): prune query rows whose id is absent from the
core's doc slab, sort the survivors by id, and cut them into blocks of 128.
Each block touches ~31 distinct ids, so only ~60 of the core's 2048 doc tokens
can match it. Those tokens are gathered per block (grouped by doc, zero-padded
to a fixed per-doc width P) giving a [128, 16*P] score tile instead of
[128, 2048] -- a ~12x reduction in matmul columns and reduce input.

The exact-match mask folds into the matmul: ids are rank-encoded per block
(dense rank over the block's id set) as two base-B digit one-hots scaled by
ALPHA=32 and appended to the bf16 reps, so

    v[r, c] = S[r, c] + 1024 * match_digits   (match_digits == 2 iff equal id)

and tok = max(v_max, OFF) - OFF with OFF=2048 reproduces the reference
masked-max (pad columns give v = S' + <=1024 < OFF, clamped to 0).

Per group of 4 tiles (one PSUM [128, 2*512] region, 2 tiles per bank):
either a direct DVE reduce_max straight from PSUM f32 + a tiny
tensor_scalar(max OFF, -OFF), or a ScalarE relu(v-OFF)->fp16 followed by a
packed fp16 DVE reduce_max. The per-query sum over rows is a selector matmul
(stationary fp16 0/1 membership matrix); CLS scores and the final 8-way max
run on host (a few thousand elements).
"""

import math
import os
import numpy as np
import ml_dtypes

Bq, Sq, Bd, Sd, D, Dc = 8, 512, 128, 128, 32, 768
NCORES = 8
BD_PER = Bd // NCORES          # 16 docs per core
ALPHA = 32.0
OFF = 2.0 * ALPHA * ALPHA      # 2048: offset of a full 2-digit rank match
GROUP = int(os.environ.get("KERNEL_GROUP", "4"))
# group g is a direct-DVE-reduce group iff g % DIRECT_PERIOD == PHASE
DIRECT_PERIOD = int(os.environ.get("KERNEL_DIRECT_PERIOD", "3"))
DIRECT_PHASE = int(os.environ.get("KERNEL_DIRECT_PHASE", "0"))
WARMUP_MMS = int(os.environ.get("KERNEL_WARMUP_MMS", "4"))

_CACHE = {}


def _bf16(x):
    return x.astype(ml_dtypes.bfloat16)


def _qry_row_mask(inputs):
    """[Bq, Sq] bool: rows that can contribute (attended, not CLS/SEP)."""
    mask = np.asarray(inputs["qry_attention_mask"], np.int64).copy()
    sep = mask.sum(axis=1) - 1
    mask[np.arange(Bq), sep] = 0
    mask[:, 0] = 0
    return mask.astype(bool)


def _supergroups(nt):
    """Final-sum groups: up to 8 tiles share one selector matmul."""
    return [range(g, min(g + 8, nt)) for g in range(0, nt, 8)]


def _prepare(inputs):
    """Build the per-core packed operands + the compile-time geometry.

    Returns (geom, in_maps) where geom is hashable and fully determines the
    Bass program; in_maps is the per-core dict of dram tensors.
    """
    qry_reps = np.asarray(inputs["qry_reps"], np.float32).reshape(-1, D)
    qry_ids = np.asarray(inputs["qry_input_ids"], np.int64).reshape(-1)
    doc_reps = np.asarray(inputs["doc_reps"], np.float32)
    doc_ids = np.asarray(inputs["doc_input_ids"], np.int64)
    row_ok = _qry_row_mask(inputs).reshape(-1)
    qpos_q = np.repeat(np.arange(Bq), Sq)

    rows_per_core = []
    for core in range(NCORES):
        sl = slice(core * BD_PER, (core + 1) * BD_PER)
        vocab = np.zeros(1000, dtype=bool)
        vocab[doc_ids[sl].reshape(-1)] = True
        rows = np.nonzero(row_ok & vocab[qry_ids])[0]
        rows = rows[np.argsort(qry_ids[rows], kind="stable")]
        rows_per_core.append(rows)
    nt = max((len(r) + 127) // 128 for r in rows_per_core)

    # per (core, tile): id set + per-doc matching token count
    idsets = [[None] * nt for _ in range(NCORES)]
    maxdist = 1
    P_ct = np.zeros((NCORES, nt), dtype=np.int64)
    for core in range(NCORES):
        dids2 = doc_ids[core * BD_PER : (core + 1) * BD_PER]
        rows = rows_per_core[core]
        for t in range(nt):
            rr = rows[t * 128 : (t + 1) * 128]
            if len(rr) == 0:
                idsets[core][t] = np.zeros(0, np.int64)
                continue
            idset = np.unique(qry_ids[rr])
            idsets[core][t] = idset
            maxdist = max(maxdist, len(idset))
            P_ct[core, t] = np.isin(dids2, idset).sum(axis=1).max()
    base = max(7, math.ceil(math.sqrt(maxdist)))
    ndig = 2 * base
    kext = D + ndig

    # group geometry (uniform across cores)
    groups = []
    for t0 in range(0, nt, GROUP):
        ntiles = min(GROUP, nt - t0)
        P = max(1, int(P_ct[:, t0 : t0 + ntiles].max()))
        ds = 1
        while (BD_PER // ds) * P > 512:
            ds *= 2
        groups.append((ntiles, P, ds))
    geom = (kext, base, nt, tuple(groups))

    # column packing
    totcol = sum(ntiles * ds * (BD_PER // ds) * P for ntiles, P, ds in groups)

    in_maps = []
    for core in range(NCORES):
        rows = rows_per_core[core]
        dreps = doc_reps[core * BD_PER : (core + 1) * BD_PER].reshape(-1, D)
        dids = doc_ids[core * BD_PER : (core + 1) * BD_PER].reshape(-1)
        dreps_bf = _bf16(dreps).astype(np.float32)
        qreps_bf = _bf16(qry_reps).astype(np.float32)

        qryT = np.zeros((kext, nt * 128), dtype=np.float32)
        docT = np.zeros((kext, totcol), dtype=np.float32)
        selT = np.zeros((128, 8 * nt), dtype=np.float32)
        col = 0
        for g, (ntiles, P, ds) in enumerate(groups):
            dps = BD_PER // ds
            for tl in range(ntiles):
                t = g * GROUP + tl
                rr = rows[t * 128 : (t + 1) * 128]
                nr = len(rr)
                idset = idsets[core][t]
                if nr:
                    rank_lookup = np.full(1000, -1, np.int64)
                    rank_lookup[idset] = np.arange(len(idset))
                    rk = rank_lookup[qry_ids[rr]]
                    c0 = t * 128
                    qryT[:D, c0 : c0 + nr] = qreps_bf[rr].T
                    qryT[D + rk % base, c0 + np.arange(nr)] = ALPHA
                    qryT[D + base + rk // base, c0 + np.arange(nr)] = ALPHA
                    selT[np.arange(nr), t * 8 + qpos_q[rr]] = 1.0
                    tokmask = np.isin(
                        dids.reshape(BD_PER, Sd), idset
                    )
                else:
                    tokmask = np.zeros((BD_PER, Sd), dtype=bool)
                # doc columns: sub-major (docs split ds ways), doc-major, pad P
                for h in range(ds):
                    for dd in range(dps):
                        d = h * dps + dd
                        js = np.nonzero(tokmask[d])[0]
                        cc = col + h * dps * P + dd * P
                        if len(js):
                            docT[:D, cc : cc + len(js)] = dreps_bf[
                                d * Sd + js
                            ].T
                            rk = rank_lookup[dids[d * Sd + js]]
                            docT[D + rk % base, cc + np.arange(len(js))] = ALPHA
                            docT[
                                D + base + rk // base, cc + np.arange(len(js))
                            ] = ALPHA
                    # half h occupies cols [col + h*dps*P, col + (h+1)*dps*P)
                col += ds * dps * P
        in_maps.append(
            {
                "qryT": _bf16(qryT),
                "docT": _bf16(docT),
                "selT": selT.astype(np.float16),
            }
        )
    return geom, in_maps


_LDW_PATCHED = False


def _patch_ldw_opt():
    """Opt-in only: append --enable-ldw-opt=true to walrus args."""
    global _LDW_PATCHED
    if _LDW_PATCHED or not os.environ.get("KERNEL_LDW_OPT"):
        return
    import concourse.bass_utils as bu

    orig = bu.get_walrus_args

    def patched(*a, **k):
        return orig(*a, **k) + ["--enable-ldw-opt=true"]

    bu.get_walrus_args = patched
    _LDW_PATCHED = True


def _split_multi_waits(nc, mybir):
    """This container's walrus accepts only ONE sync-wait per instruction.
    Hoist extra waits into standalone EventSemaphore instructions on the same
    engine right before the offender (sequencer blocks on each in order)."""
    n = 0
    for func in nc.m.functions:
        for bb in func.blocks:
            out = []
            for inst in bb.instructions:
                si = inst.sync_info
                if si is not None and len(si.on_wait) > 1:
                    waits = list(si.on_wait)
                    for w in waits[:-1]:
                        n += 1
                        out.append(
                            mybir.InstEventSemaphore(
                                name=f"W-{inst.name}-{n}",
                                engine=inst.engine,
                                ins=[],
                                outs=[],
                                debug=inst.debug,
                                sync_info=mybir.SyncInfo(
                                    on_wait=[w], on_update=[]
                                ),
                            )
                        )
                    inst.sync_info = mybir.SyncInfo(
                        on_wait=[waits[-1]], on_update=list(si.on_update)
                    )
                out.append(inst)
            bb.instructions = out
    return n


def _build_nc(geom):
    import concourse.bass as bass
    import concourse.mybir as mybir
    import concourse.tile as tile

    kext, base, nt, groups = geom
    bf16, f16, f32 = mybir.dt.bfloat16, mybir.dt.float16, mybir.dt.float32
    nc = bass.Bass("TRN2", target_bir_lowering=False, debug=False)

    # per-group packing info
    ginfo = []  # (t0, ntiles, P, ds, dps, Ws, per_bank, nb, colofs, gcols)
    col = 0
    nb_max = 1
    for g, (ntiles, P, ds) in enumerate(groups):
        dps = BD_PER // ds
        Ws = dps * P
        nsubs = ntiles * ds
        per_bank = max(1, 512 // Ws) if ds == 1 else 1
        nb = (nsubs + per_bank - 1) // per_bank
        nb_max = max(nb_max, nb)
        gcols = nsubs * Ws
        ginfo.append((g * GROUP, ntiles, P, ds, dps, Ws, per_bank, nb, col, gcols))
        col += gcols
    totcol = col

    qryT = nc.dram_tensor("qryT", [kext, nt * 128], bf16, kind="ExternalInput").ap()
    docT = nc.dram_tensor("docT", [kext, totcol], bf16, kind="ExternalInput").ap()
    selT = nc.dram_tensor("selT", [128, 8 * nt], f16, kind="ExternalInput").ap()
    out = nc.dram_tensor("out", [64, 16 * nt], f16, kind="ExternalOutput").ap()

    n_groups = len(ginfo)
    is_direct = [
        DIRECT_PERIOD > 0 and g % DIRECT_PERIOD == DIRECT_PHASE % DIRECT_PERIOD
        for g in range(n_groups)
    ]

    with tile.TileContext(nc) as tc:
        with (
            tc.tile_pool(name="inp", bufs=1) as inp,
            tc.tile_pool(name="psum", bufs=3, space="PSUM") as psum,
            tc.tile_pool(name="stage", bufs=2) as stp,
            tc.tile_pool(name="accp", bufs=1) as accp,
        ):
            # input SBUF + DMA: group-by-group so tile 0 can start early.
            qry_sb = inp.tile([kext, nt * 128], bf16)
            doc_sb = inp.tile([kext, totcol], bf16)
            sel_sb = inp.tile([128, 8 * nt], f16)
            for gi, (t0, ntiles, P, ds, dps, Ws, per_bank, nb, colofs, gcols) in (
                enumerate(ginfo)
            ):
                qa, qb = t0 * 128, (t0 + ntiles) * 128
                e_doc = nc.sync if gi % 2 == 0 else nc.gpsimd
                e_qry = nc.gpsimd if gi % 2 == 0 else nc.sync
                e_doc.dma_start(
                    doc_sb[:, colofs : colofs + gcols],
                    docT[:, colofs : colofs + gcols],
                )
                e_qry.dma_start(qry_sb[:, qa:qb], qryT[:, qa:qb])
            nc.sync.dma_start(sel_sb[:], selT[:])

            negoff = accp.tile([128, 1], f32)
            nc.vector.memset(negoff[:], -OFF)
            # tiny dummy activation: pulls the Relu ACT_TABLE_LOAD into the
            # DMA head instead of stalling the first real group
            atl = accp.tile([128, 1], f16)
            nc.scalar.activation(
                atl[:], negoff[:], mybir.ActivationFunctionType.Relu,
                bias=negoff[:],
            )

            # PE warm-up during the DMA head (HAM clock ramp)
            if WARMUP_MMS:
                scratch = inp.tile([kext, 512], bf16)
                nc.vector.memset(scratch[:], 0.0)
                wps = psum.tile([128, 512], f32, tag="score")
                for _ in range(WARMUP_MMS):
                    nc.tensor.matmul(
                        wps[:], scratch[:, 0:128], scratch[:],
                        start=True, stop=True,
                    )

            accum = accp.tile([128, 16 * nt], f16)
            draw = accp.tile([128, 16 * nt], f32)

            for gi, (t0, ntiles, P, ds, dps, Ws, per_bank, nb, colofs, gcols) in (
                enumerate(ginfo)
            ):
                nsubs = ntiles * ds
                ps = psum.tile([128, nb * 512], f32, tag="score")
                # matmuls: sub j -> bank j//per_bank, slot (j%per_bank)*Ws
                for j in range(nsubs):
                    t = t0 + j // ds
                    slot = (j // per_bank) * 512 + (j % per_bank) * Ws
                    sub = colofs + j * Ws
                    nc.tensor.matmul(
                        ps[:, slot : slot + Ws],
                        qry_sb[:, t * 128 : (t + 1) * 128],
                        doc_sb[:, sub : sub + Ws],
                        start=True,
                        stop=True,
                    )

                # PSUM view chunks: (flat_view [p,(banks,)subs,Ws],
                #                    grouped_view [...,d,t], n_subs_in_chunk)
                def psum_chunks():
                    chunks = []
                    if ds == 1:
                        nfull = nsubs // per_bank
                        rem = nsubs % per_bank
                        if nfull:
                            flat = ps[:, 0 : nfull * 512].rearrange(
                                "p (nb c) -> p nb c", c=512
                            )[:, :, 0 : per_bank * Ws].rearrange(
                                "p nb (s c) -> p nb s c", c=Ws
                            )
                            grp = ps[:, 0 : nfull * 512].rearrange(
                                "p (nb c) -> p nb c", c=512
                            )[:, :, 0 : per_bank * Ws].rearrange(
                                "p nb (s d t) -> p nb s d t", d=dps, t=P
                            )
                            chunks.append((flat, grp, nfull * per_bank))
                        if rem:
                            lo = nfull * 512
                            flat = ps[:, lo : lo + rem * Ws].rearrange(
                                "p (s c) -> p s c", c=Ws
                            )
                            grp = ps[:, lo : lo + rem * Ws].rearrange(
                                "p (s d t) -> p s d t", d=dps, t=P
                            )
                            chunks.append((flat, grp, rem))
                    else:
                        # one sub per bank; banks factor as (tile, half)
                        flat = ps[:, 0 : nsubs * 512].rearrange(
                            "p (nt h c) -> p nt h c", h=ds, c=512
                        )[:, :, :, 0:Ws]
                        grp = ps[:, 0 : nsubs * 512].rearrange(
                            "p (nt h c) -> p nt h c", h=ds, c=512
                        )[:, :, :, 0:Ws].rearrange(
                            "p nt h (d t) -> p nt h d t", t=P
                        )
                        chunks.append((flat, grp, nsubs))
                    return chunks

                c0 = t0 * 16
                if is_direct[gi]:
                    dcol = c0
                    for flat, grp, nsub_c in psum_chunks():
                        ncols = nsub_c * dps
                        od = draw[:, dcol : dcol + ncols]
                        if len(grp.shape) == 5:
                            od = od.rearrange(
                                "p (a s d) -> p a s d", d=dps, s=grp.shape[2]
                            )
                        else:
                            od = od.rearrange("p (s d) -> p s d", d=dps)
                        nc.vector.reduce_max(od, grp, axis=mybir.AxisListType.X)
                        dcol += ncols
                    nc.vector.tensor_scalar(
                        accum[:, c0 : c0 + 16 * ntiles],
                        draw[:, c0 : c0 + 16 * ntiles],
                        OFF,
                        -OFF,
                        mybir.AluOpType.max,
                        mybir.AluOpType.add,
                    )
                else:
                    st = stp.tile([128, nsubs * Ws], f16, tag="stage")
                    scol = 0
                    for flat, grp, nsub_c in psum_chunks():
                        w = nsub_c * Ws
                        so = st[:, scol : scol + w]
                        if len(flat.shape) == 4:
                            so = so.rearrange(
                                "p (nb s c) -> p nb s c",
                                nb=flat.shape[1], c=Ws,
                            )
                        else:
                            so = so.rearrange("p (s c) -> p s c", c=Ws)
                        nc.scalar.activation(
                            so, flat,
                            mybir.ActivationFunctionType.Relu,
                            bias=negoff[:],
                        )
                        scol += w
                    nc.vector.reduce_max(
                        accum[:, c0 : c0 + 16 * ntiles].rearrange(
                            "p (a d) -> p a d", d=dps
                        ),
                        st[:].rearrange("p (a d t) -> p a d t", d=dps, t=P),
                        axis=mybir.AxisListType.X,
                    )

            # per-q partition sums: selector matmul per supergroup of 8 tiles
            osb = accp.tile([64, 16 * nt], f16)
            for g, grp in enumerate(_supergroups(nt)):
                qts = list(grp)
                gn = len(qts)
                c0 = qts[0] * 16
                fin = psum.tile([8 * gn, 16 * gn], f32, tag="score")
                nc.tensor.matmul(
                    fin[:],
                    sel_sb[:, qts[0] * 8 : (qts[-1] + 1) * 8],
                    accum[:, c0 : c0 + 16 * gn],
                    start=True,
                    stop=True,
                )
                if g % 2 == 0:
                    nc.vector.tensor_copy(osb[0 : 8 * gn, c0 : c0 + 16 * gn], fin[:])
                else:
                    nc.scalar.copy(osb[0 : 8 * gn, c0 : c0 + 16 * gn], fin[:])
            nc.sync.dma_start(out[:], osb[:])
    _split_multi_waits(nc, mybir)
    return nc


def _get_nc(geom):
    _patch_ldw_opt()
    key = (geom, GROUP, DIRECT_PERIOD, DIRECT_PHASE, WARMUP_MMS)
    if key not in _CACHE:
        _CACHE[key] = _build_nc(geom)
    return _CACHE[key]


def _assemble(inputs, results, nt):
    toks = np.zeros((Bq, Bd), dtype=np.float32)
    for core in range(NCORES):
        osb = np.asarray(results[core]["out"], np.float32)  # [64, 16*nt]
        part = np.zeros((Bq, BD_PER), dtype=np.float32)
        for grp in _supergroups(nt):
            for tl, t in enumerate(grp):
                part += osb[8 * tl : 8 * tl + 8, t * 16 : (t + 1) * 16]
        toks[:, core * BD_PER : (core + 1) * BD_PER] = part
    cls = np.asarray(inputs["qry_cls"], np.float32) @ np.asarray(
        inputs["doc_cls"], np.float32
    ).T
    scores = toks + cls
    return scores.max(axis=0).reshape(-1).astype(np.float32)


def _ensure_ntff_hook():
    """This container's antenv lacks axon_hooks; synthesize the module and
    register the ctypes-based NTFF profile hook so trace=True works."""
    import sys
    import types

    if "antenv.axon_hooks" in sys.modules:
        return
    mod = types.ModuleType("antenv.axon_hooks")
    state = {"hook": None}
    mod.set_axon_ntff_profile_hook = lambda h: state.__setitem__("hook", h)
    mod.get_axon_ntff_profile_hook = lambda: state["hook"]
    sys.modules["antenv.axon_hooks"] = mod
    try:
        import antenv

        antenv.axon_hooks = mod
    except ImportError:
        pass
    try:
        from trn_agent_boot.trn_boot import _ntff_profile_via_ctypes

        mod.set_axon_ntff_profile_hook(
            _ntff_profile_via_ctypes("/opt/axon/libaxon_pjrt.so")
        )
    except Exception:
        pass


def run(inputs, trace=False, **kwargs):
    """Run on the 8 NeuronCores; returns (output, BassKernelResults)."""
    from concourse.bass_utils import run_bass_kernel_spmd

    if trace:
        _ensure_ntff_hook()
    geom, in_maps = _prepare(inputs)
    nc = _get_nc(geom)
    res = run_bass_kernel_spmd(
        nc, in_maps, core_ids=list(range(NCORES)), trace=trace, **kwargs
    )
    return _assemble(inputs, res.results, geom[2]), res


def kernel(**inputs) -> np.ndarray:
    out, _ = run(inputs)
    return out
